# revision 91
# baseline (speedup 1.0000x reference)
"""Trainium2 Bass kernel for ClassForgeEnsembleGNN (SAGE -> GAT -> RGCN ensemble).

Strategy (8 NeuronCores, SPMD):
  - Nodes partitioned into 8 contiguous shards (6250 each); each core owns the
    edges whose target is in its shard.  Weights replicated; x1/x2 node
    features all-gathered between stages (device collectives).
  - Per-edge source rows fetched with batched GPSIMD dma_gather (int16
    indices; tables addressed through two views split at 32768).
  - Scatter-aggregation via selection-matrix matmuls where the selection
    matrices S[e, n] (0/1, with 1/deg mean weights folded in bf16) are
    PRECOMPUTED ON HOST and streamed from HBM in 512KB group DMAs -- no
    on-device S construction.
  - SAGE/RGCN aggregate feature-major (lhsT=gathered rows, rhs=S) so the
    result lands [d, n] ready for the weight matmul without a transpose.
  - GAT: self-loops appended as real edges; per-edge a_d via a transposed
    selection matmul (S_T also precomputed); exp(leaky(logit)) folded into
    S at runtime by one DVE scale per head; denominator via a ones column.
  - RGCN: relation runs inside tiles use per-run masked S; all 5 relation
    terms + root accumulate into one PSUM tile per block.
"""

import sys
import os

for _p in ("/opt/trn_rl_repo", "/root/.axon_site/_ro/trn_rl_repo"):
    if os.path.isdir(_p) and _p not in sys.path:
        sys.path.append(_p)

import numpy as np
import ml_dtypes

import concourse.bacc as bacc
import concourse.bass as bass
import concourse.mybir as mybir
import concourse.tile as tile
from concourse.bass_utils import run_bass_kernel_spmd
from concourse.masks import make_identity

P = 128
NCORES = 8
N = 50000
E = 400000
D = 128
H = 2
R = 5
NEG = 0.2
SH = N // NCORES            # 6250 nodes per shard
B = (SH + P - 1) // P       # 49 blocks (last has 106 valid nodes)
HSPLIT = 32768              # int16-safe table split
BCUT = 32                   # blocks in collective chunk A (8*CH = 32768 = int16 limit)
CH = BCUT * P               # 3200 rows per shard in chunk A
CB = SH - CH                # 3050 rows in chunk B
G_GATHER = 8                # tiles per dma_gather group
G_S = 16                    # tiles per S-matrix group DMA

f32 = mybir.dt.float32
bf16 = mybir.dt.bfloat16
i32 = mybir.dt.int32
i16 = mybir.dt.int16
AF = mybir.ActivationFunctionType
ALU = mybir.AluOpType
BF = ml_dtypes.bfloat16


def _pack_idx(idx, T):
    """flat slot->src indices [T*128] -> dma_gather int16 layout [128, T*8]."""
    m = idx.reshape(-1, 16).T.astype(np.int16)
    return np.ascontiguousarray(np.tile(m, (8, 1)))


def _to_pm(S_flat, T):
    """[T*128, 128] -> partition-major [128, T*128] (tile t at cols t*128:)."""
    return np.ascontiguousarray(
        S_flat.reshape(T, P, P).transpose(1, 0, 2).reshape(P, T * P))


def _to_pm_g(S_flat, T, G):
    """[T*128, 128] -> group-contiguous [nG, 128, G*128]: each group's
    partition-major block is one linear DRAM region (max DMA bandwidth)."""
    nG = (T + G - 1) // G
    S = np.zeros((nG * G * P, P), S_flat.dtype)
    S[:T * P] = S_flat
    return np.ascontiguousarray(
        S.reshape(nG, G, P, P).transpose(0, 2, 1, 3).reshape(nG, P, G * P))


def _pack_stage(half_pc, tidx_pc, src_pc, dst_pc, w_pc, rel_pc, nrel):
    """Group edges by (half, dst_block [, rel]); sections (h, b) tile-aligned,
    rel runs slot-aligned; uniform tile/slot caps across cores.

    half_pc: per-edge stream id (0/1); tidx_pc: per-edge gather-table index
    within its stream.
    """
    counts = np.zeros((NCORES, 2, B, nrel), np.int64)
    for k in range(NCORES):
        h = half_pc[k].astype(np.int64)
        blk = dst_pc[k] // P
        r = rel_pc[k] if nrel > 1 else np.zeros(len(src_pc[k]), np.int64)
        np.add.at(counts[k], (h, blk, r), 1)
    caps = counts.max(0)                       # [2, B, nrel]
    run_off = np.zeros((2, B, nrel + 1), np.int64)
    np.cumsum(caps, axis=2, out=run_off[:, :, 1:])
    sec_slots = run_off[:, :, nrel]
    sec_tiles = (sec_slots + P - 1) // P
    tile_off = np.zeros((2, B + 1), np.int64)
    np.cumsum(sec_tiles, axis=1, out=tile_off[:, 1:])
    T = (int(tile_off[0, B]), int(tile_off[1, B]))

    # runs: list per h of (b, tile_in_stream, r, slot_lo, slot_hi) in
    # execution order (b asc, tile asc, r asc); only for nrel > 1
    runs = ([], [])
    run_idx = {}
    if nrel > 1:
        for b in range(B):
            for h in range(2):
                t0 = int(tile_off[h, b])
                for tl in range(int(sec_tiles[h, b])):
                    s0, s1 = tl * P, (tl + 1) * P
                    for r in range(nrel):
                        lo = int(run_off[h, b, r])
                        hi = int(run_off[h, b, r + 1])
                        if lo < s1 and hi > s0:
                            run_idx[(h, t0 + tl, r)] = len(runs[h])
                            runs[h].append((b, t0 + tl, r,
                                            max(lo, s0) - s0,
                                            min(hi, s1) - s0))

    cores = []
    for k in range(NCORES):
        src = src_pc[k]
        tidx = tidx_pc[k]
        dst = dst_pc[k]
        w = w_pc[k]
        h = half_pc[k].astype(np.int64)
        blk = dst // P
        r = rel_pc[k] if nrel > 1 else np.zeros(len(src), np.int64)
        gid = (h * B + blk) * nrel + r
        order = np.argsort(gid * np.int64(N) + src, kind="stable")
        gs = gid[order]
        cnt_flat = counts[k].reshape(-1)
        starts = np.concatenate([[0], np.cumsum(cnt_flat)])[:-1]
        rank = np.arange(len(gs)) - starts[gs]
        hh, rest = gs // (B * nrel), gs % (B * nrel)
        bb, rr = rest // nrel, rest % nrel
        slot = tile_off[hh, bb] * P + run_off[hh, bb, rr] + rank
        per_h = []
        for hv in range(2):
            Th = T[hv]
            n_slots = Th * P
            sel = hh == hv
            s = slot[sel]
            idx = np.zeros(n_slots, np.int64)
            idx[s] = tidx[order][sel]
            srcs = np.zeros(n_slots, np.int64)
            srcs[s] = src[order][sel]
            ws = np.zeros(n_slots, np.float32)
            ws[s] = w[order][sel]
            dcol = np.full(n_slots, -1.0, np.float32)
            dcol[s] = (dst[order][sel] % P).astype(np.float32)
            if nrel > 1:
                # per-run weight column: w inside the run's slot range,
                # 0 elsewhere (masks the tile-shared 0/1 S to this run)
                NR = len(runs[hv])
                rwc = np.zeros((NR, P), np.float32)
                for j, (b_, t_, r_, lo_, hi_) in enumerate(runs[hv]):
                    rwc[j, lo_:hi_] = ws[t_ * P + lo_:t_ * P + hi_]
                rwcol = np.ascontiguousarray(rwc.T)
            else:
                rwcol = None
            S_flat = np.zeros((n_slots, P), np.float32)
            S_flat[s, (dst[order][sel] % P)] = w[order][sel]
            per_h.append(dict(idx16=_pack_idx(idx, Th), srcs=srcs,
                              ws=ws,
                              dcol=np.ascontiguousarray(
                                  dcol.reshape(Th, P).T) if Th else None,
                              rwcol=rwcol,
                              S_flat=S_flat.astype(BF) if nrel == 1
                              else None))
        cores.append(per_h)

    return dict(T=T, tile_off=tile_off, sec_tiles=sec_tiles, runs=runs,
                run_idx=run_idx, cores=cores)


def _preprocess(x, edge_index, edge_type):
    src = edge_index[0].astype(np.int64)
    dst = edge_index[1].astype(np.int64)
    et = edge_type.astype(np.int64)

    cnt = np.bincount(dst, minlength=N).astype(np.float32)
    wrec = (1.0 / np.maximum(cnt, 1.0)).astype(np.float32)
    cnt_r = np.zeros((R, N), np.float32)
    for r in range(R):
        cnt_r[r] = np.bincount(dst[et == r], minlength=N)
    wrrec = (1.0 / np.maximum(cnt_r, 1.0)).astype(np.float32)

    def chunk_of(s):
        """Stream (0=A,1=B) and table index for the chunked AllGather
        layout: chunk A = first CH rows of each shard, B = the rest."""
        sh, loc = s // SH, s % SH
        half = (loc >= CH).astype(np.int64)
        tidx = np.where(half == 0, sh * CH + loc, sh * CB + (loc - CH))
        return half, tidx

    shard_of = dst // SH
    s_src, s_dst, s_rel, s_w, s_wr, s_h, s_t = [], [], [], [], [], [], []
    g_src, g_dst, g_h, g_t = [], [], [], []
    for k in range(NCORES):
        sel = shard_of == k
        s_src.append(src[sel])
        s_dst.append(dst[sel] - k * SH)
        s_rel.append(et[sel])
        s_w.append(wrec[dst[sel]])
        s_wr.append(wrrec[et[sel], dst[sel]])
        s_h.append(np.zeros(sel.sum(), np.int64))
        s_t.append(src[sel])
        hh, tt = chunk_of(src[sel])
        # GAT: append self loops (weight 1.0 folded at runtime with exp)
        loc = np.arange(SH, dtype=np.int64)
        gsrc = np.concatenate([src[sel], loc + k * SH])
        g_src.append(gsrc)
        g_dst.append(np.concatenate([dst[sel] - k * SH, loc]))
        ghh, gtt = chunk_of(gsrc)
        g_h.append(ghh)
        g_t.append(gtt)

    sage = _pack_stage(s_h, s_t, s_src, s_dst, s_w,
                       [None] * NCORES, 1)
    gat = _pack_stage(g_h, g_t, g_src, g_dst,
                      [np.ones(len(g_src[k]), np.float32)
                       for k in range(NCORES)],
                      [None] * NCORES, 1)
    rgcn = _pack_stage([chunk_of(s)[0] for s in s_src],
                       [chunk_of(s)[1] for s in s_src],
                       s_src, s_dst, s_wr, s_rel, R)

    # GAT transposed selection (0/1): ST[t][n, e] = S[t][e, n]
    for k in range(NCORES):
        for hv in range(2):
            Th = gat["T"][hv]
            Sf = gat["cores"][k][hv]["S_flat"]       # [T*128, 128]
            STf = Sf.reshape(Th, P, P).transpose(0, 2, 1).reshape(Th * P, P)
            gat["cores"][k][hv]["ST_flat"] = STf

    return dict(sage=sage, gat=gat, rgcn=rgcn)


def _build_program(pp):
    sage, gat, rgcn = pp["sage"], pp["gat"], pp["rgcn"]

    nc = bacc.Bacc("TRN2", target_bir_lowering=False, debug=False,
                   num_devices=NCORES, num_swdge_queues=4)

    xt_dram = nc.dram_tensor("xt", [B, P, P], bf16, kind="ExternalInput")
    meta = {}
    for st, d_ in (("e", sage), ("g", gat), ("r", rgcn)):
        for h in range(2):
            T = d_["T"][h]
            if T == 0:
                continue
            if st == "e":
                # SAGE source rows pre-gathered on host (x is an input),
                # with the 1/deg mean weight folded in; S built on device
                # from the dst column.
                meta[f"ex{h}"] = nc.dram_tensor(
                    f"ex{h}", [(T + 15) // 16, P, 16 * P], bf16,
                    kind="ExternalInput")
                meta[f"ed{h}"] = nc.dram_tensor(
                    f"ed{h}", [P, T], f32, kind="ExternalInput")
            elif st == "g":
                meta[f"gi{h}"] = nc.dram_tensor(
                    f"gi{h}", [P, T * 8], i16, kind="ExternalInput")
                meta[f"gs{h}"] = nc.dram_tensor(
                    f"gs{h}", [(T + G_S - 1) // G_S, P, G_S * P], bf16,
                    kind="ExternalInput")
                meta[f"gq{h}"] = nc.dram_tensor(
                    f"gq{h}", [(T + G_S - 1) // G_S, P, G_S * P], bf16,
                    kind="ExternalInput")
            else:
                meta[f"ri{h}"] = nc.dram_tensor(
                    f"ri{h}", [P, T * 8], i16, kind="ExternalInput")
                meta[f"rd{h}"] = nc.dram_tensor(
                    f"rd{h}", [P, T], f32, kind="ExternalInput")
                meta[f"rw{h}"] = nc.dram_tensor(
                    f"rw{h}", [P, len(d_["runs"][h])], f32,
                    kind="ExternalInput")
                meta[f"rb{h}"] = nc.dram_tensor(
                    f"rb{h}", [P, len(d_["runs"][h])], bf16,
                    kind="ExternalInput")
    wsl = nc.dram_tensor("wsl", [D, D], bf16, kind="ExternalInput")
    wsr = nc.dram_tensor("wsr", [D, D], bf16, kind="ExternalInput")
    bs = nc.dram_tensor("bs", [P, 1], f32, kind="ExternalInput")
    vsd = nc.dram_tensor("vsd", [D, 4], f32, kind="ExternalInput")
    wg0 = nc.dram_tensor("wg0", [D, D], bf16, kind="ExternalInput")
    wg1 = nc.dram_tensor("wg1", [D, D], bf16, kind="ExternalInput")
    bg = nc.dram_tensor("bg", [P, 1], f32, kind="ExternalInput")
    wroot = nc.dram_tensor("wroot", [D, D], bf16, kind="ExternalInput")
    wr_d = nc.dram_tensor("wr", [R, D, D], bf16, kind="ExternalInput")
    br = nc.dram_tensor("br", [P, 1], f32, kind="ExternalInput")
    out_dram = nc.dram_tensor("out", [SH, 3 * D], f32, kind="ExternalOutput")

    rg = [list(range(NCORES))]
    qrr = [0]

    def next_q():
        q = qrr[0]
        qrr[0] = (q + 1) % 4
        return q

    with tile.TileContext(nc) as tc:
        with (
            tc.tile_pool(name="const", bufs=1) as cb,
            tc.tile_pool(name="sbuf", bufs=2) as sb,
            tc.tile_pool(name="psum", bufs=1, space="PSUM") as ps,
            tc.tile_pool(name="dram", bufs=1, space="DRAM") as dr,
        ):
            ident = cb.tile([P, P], f32)
            make_identity(nc, ident[:])
            identb = cb.tile([P, P], bf16)
            nc.vector.tensor_copy(identb[:], ident[:])
            iota_i = cb.tile([P, P], i32)
            nc.gpsimd.iota(iota_i[:], pattern=[[1, P]], base=0,
                           channel_multiplier=0)
            iota_f = cb.tile([P, P], f32)
            nc.vector.tensor_copy(iota_f[:], iota_i[:])

            def load_const(name, dram, dtype):
                t = cb.tile(list(dram.shape), dtype, name=name)
                nc.sync.dma_start(t[:], dram[:])
                return t

            wsl_sb = load_const("wsl_sb", wsl, bf16)
            wsr_sb = load_const("wsr_sb", wsr, bf16)
            bs_sb = load_const("bs_sb", bs, f32)
            vsd_sb = load_const("vsd_sb", vsd, f32)
            wg0_sb = load_const("wg0_sb", wg0, bf16)
            wg1_sb = load_const("wg1_sb", wg1, bf16)
            bg_sb = load_const("bg_sb", bg, f32)
            wroot_sb = load_const("wroot_sb", wroot, bf16)
            br_sb = load_const("br_sb", br, f32)
            wr_sb = cb.tile([P, R * D], bf16)
            for r in range(R):
                nc.sync.dma_start(wr_sb[:, r * D:(r + 1) * D], wr_d[r, :, :])
            idx_sb = {}
            for name, dram in meta.items():
                if name[1] not in "idwb" or name == "xb":
                    continue
                t = cb.tile(list(dram.shape), dram.dtype, name=f"{name}_sb")
                nc.sync.dma_start(t[:], dram[:])
                idx_sb[name] = t

            # persistent per-shard state
            adb_sb = cb.tile([P, 2 * B], bf16)   # a_d bf16 per block
            x2Tb_sb = cb.tile([P, B * P], bf16)  # x2 feature-major bf16

            cc1_in = [dr.tile([CH, 256], bf16, name="cc1a_in"),
                      dr.tile([CB, 256], bf16, name="cc1b_in")]
            cc1_out = [dr.tile([NCORES * CH, 256], bf16,
                               addr_space="Shared", name="cc1a_out"),
                       dr.tile([NCORES * CB, 256], bf16,
                               addr_space="Shared", name="cc1b_out")]
            cc2_in = [dr.tile([CH, D], bf16, name="cc2a_in"),
                      dr.tile([CB, D], bf16, name="cc2b_in")]
            cc2_out = [dr.tile([NCORES * CH, D], bf16,
                               addr_space="Shared", name="cc2a_out"),
                       dr.tile([NCORES * CB, D], bf16,
                               addr_space="Shared", name="cc2b_out")]

            one3 = cb.tile([P, 1], bf16)
            nc.vector.memset(one3[:], 1.0)
            negc = cb.tile([P, 1], f32)
            nc.vector.memset(negc[:], NEG)

            # ---------- streamed gather groups (prefetch 1 ahead) ----------
            def make_gather(st, d_, table_views, width):
                state = {}

                def load(h, g):
                    key = (h, g)
                    if key in state:
                        return
                    T = d_["T"][h]
                    g0 = g * G_GATHER
                    if g0 >= T:
                        return
                    gn = min(G_GATHER, T - g0)
                    xg = sb.tile([P, G_GATHER, width], bf16,
                                 tag=f"{st}xg{h}", bufs=5)
                    ni = gn * P
                    nc.gpsimd.dma_gather(
                        xg[:, 0:gn, :], table_views[h],
                        idx_sb[f"{st}i{h}"][:, g0 * 8:(g0 + gn) * 8],
                        ni, ni, width, queue_num=next_q())
                    state[key] = xg

                def get_tile(h, t):
                    g = t // G_GATHER
                    for pf in range(4):
                        load(h, g + pf)
                    return state[(h, g)][:, t - g * G_GATHER, :]

                return get_tile

            # ---------- streamed S-matrix groups ----------
            def make_sstream(name, nmax_by_h, gs=G_S, bufs=2, pf=1):
                state = {}

                def load(h, g):
                    key = (h, g)
                    if key in state:
                        return
                    nmax = nmax_by_h[h]
                    g0 = g * gs
                    if g0 >= nmax:
                        return
                    gn = min(gs, nmax - g0)
                    t_ = sb.tile([P, gs * P], bf16, tag=f"{name}{h}",
                                 bufs=bufs)
                    nc.scalar.dma_start(
                        t_[:, 0:gn * P],
                        meta[f"{name}{h}"][g, :, 0:gn * P])
                    state[key] = t_

                def get(h, j, span=1):
                    g = j // gs
                    for q in range(pf + 1):
                        load(h, g + q)
                    j0 = j - g * gs
                    return state[(h, g)][:, j0 * P:(j0 + span) * P]

                return get

            # =================== Stage 1: SAGE ===================
            sc = nc.enter_named_scope("sage", False)
            sage_xg = make_sstream("ex", sage["T"], gs=16, bufs=6, pf=4)
            for b in range(B):
                vld = min(P, SH - b * P)
                r0 = b * P
                tiles = [(0, t) for t in range(int(sage["tile_off"][0, b]),
                                               int(sage["tile_off"][0, b + 1]))]
                pa = ps.tile([P, P], f32, tag="accA", bufs=2)
                if tiles:
                    for j, (h, t) in enumerate(tiles):
                        xg = sage_xg(h, t)
                        Sw = sb.tile([P, P], bf16, tag="Ssage", bufs=4)
                        nc.vector.tensor_tensor(
                            out=Sw[:], in0=iota_f[:],
                            in1=idx_sb["ed0"][:, t:t + 1].to_broadcast(
                                [P, P]),
                            op=ALU.is_equal)
                        nc.tensor.matmul(pa[:], lhsT=xg, rhs=Sw[:],
                                         start=(j == 0),
                                         stop=(j == len(tiles) - 1))
                else:
                    nc.vector.memset(pa[:], 0.0)
                aggTb = sb.tile([P, P], bf16, tag="aggTb")
                nc.scalar.copy(aggTb[:], pa[:])
                xT = sb.tile([P, P], bf16, tag="xT")
                nc.scalar.dma_start(xT[:], xt_dram[b, :, :])
                pm = ps.tile([P, P], f32, tag="mm", bufs=2)
                nc.tensor.matmul(pm[:], lhsT=wsl_sb[:], rhs=aggTb[:],
                                 start=True, stop=False)
                nc.tensor.matmul(pm[:], lhsT=wsr_sb[:], rhs=xT[:],
                                 start=False, stop=True)
                x1T = sb.tile([P, P], f32, tag="x1T")
                nc.scalar.activation(x1T[:], pm[:], AF.Relu,
                                     bias=bs_sb[:, 0:1], scale=1.0)
                pasd = ps.tile([P, 4], f32, tag="px", bufs=1)
                nc.tensor.matmul(pasd[:], lhsT=x1T[:], rhs=vsd_sb[:],
                                 start=True, stop=True)
                nc.scalar.copy(adb_sb[:, 2 * b:2 * b + 2], pasd[:, 2:4])
                ase = sb.tile([P, 3], bf16, tag="ase")
                nc.vector.tensor_copy(ase[:, 0:1], one3[:])
                nc.vector.tensor_copy(ase[:, 1:3], pasd[:, 0:2])
                ptr = ps.tile([P, P], f32, tag="mm", bufs=2)
                nc.tensor.transpose(ptr[:], x1T[:], ident[:])
                x1n = sb.tile([P, P], f32, tag="x1n")
                nc.scalar.copy(x1n[:], ptr[:])
                x1nb = sb.tile([P, P], bf16, tag="x1nb")
                nc.vector.tensor_copy(x1nb[:], ptr[:])
                hc, rc = (0, r0) if b < BCUT else (1, r0 - CH)
                nc.sync.dma_start(cc1_in[hc][rc:rc + vld, 0:D],
                                  x1nb[:vld, :])
                nc.sync.dma_start(cc1_in[hc][rc:rc + vld, D:D + 3],
                                  ase[:vld, :])
                nc.sync.dma_start(out_dram[r0:r0 + vld, 0:D], x1n[:vld, :])
                if b == BCUT - 1:
                    nc.gpsimd.collective_compute(
                        "AllGather", ALU.bypass, replica_groups=rg,
                        ins=[cc1_in[0][:]], outs=[cc1_out[0][:]])
            nc.leave_named_scope("sage", sc[0], False)

            # =================== Stage 2: GAT ===================
            sc = nc.enter_named_scope("gat", False)
            gat_S = make_sstream("gs", gat["T"])
            gat_ST = make_sstream("gq", gat["T"])

            # block of each gat tile (for adb lookup at group level)
            gat_blk = [np.zeros(max(gat["T"][h], 1), np.int64)
                       for h in range(2)]
            for h in range(2):
                for b in range(B):
                    gat_blk[h][int(gat["tile_off"][h, b]):
                               int(gat["tile_off"][h, b + 1])] = b

            # GAT gather groups: fetch rows + compute per-edge exp(leaky)
            # attention factors for the whole group in O(1) instructions.
            gdma = {}
            gstate = {}

            def gat_dma(h, g):
                key = (h, g)
                if key in gdma:
                    return
                T = gat["T"][h]
                g0 = g * G_GATHER
                if g0 >= T:
                    return
                gn = min(G_GATHER, T - g0)
                xg = sb.tile([P, G_GATHER, 256], bf16,
                             tag=f"gxg{h}", bufs=5)
                ni = gn * P
                nc.gpsimd.dma_gather(
                    xg[:, 0:gn, :], cc1_out[h][:],
                    idx_sb[f"gi{h}"][:, g0 * 8:(g0 + gn) * 8],
                    ni, ni, 256, queue_num=next_q())
                gdma[key] = (xg, gn)

            def gat_chain(h, g):
                key = (h, g)
                if key in gstate:
                    return
                T = gat["T"][h]
                g0 = g * G_GATHER
                if g0 >= T:
                    return
                gat_dma(h, g)
                xg, gn = gdma[key]
                padg = ps.tile([P, G_GATHER, 2], f32, tag="accC", bufs=2)
                for jg in range(gn):
                    bb = int(gat_blk[h][g0 + jg])
                    nc.tensor.matmul(padg[:, jg, :],
                                     lhsT=gat_ST(h, g0 + jg),
                                     rhs=adb_sb[:, 2 * bb:2 * bb + 2],
                                     start=True, stop=True)
                asg = sb.tile([P, G_GATHER, 2], f32, tag=f"asg{h}", bufs=4)
                nc.scalar.copy(asg[:, 0:gn, :], xg[:, 0:gn, 129:131])
                ev = sb.tile([P, G_GATHER, 2], f32, tag=f"ev{h}", bufs=4)
                nc.vector.tensor_add(ev[:, 0:gn, :], asg[:, 0:gn, :],
                                     padg[:, 0:gn, :])
                ev2 = sb.tile([P, G_GATHER, 2], f32, tag=f"ev2{h}", bufs=4)
                nc.vector.tensor_tensor(
                    out=ev2[:, 0:gn, :], in0=ev[:, 0:gn, :],
                    in1=negc[:, 0:1].to_broadcast([P, gn, 2]),
                    op=ALU.mult)
                lr = sb.tile([P, G_GATHER, 2], f32, tag=f"lr{h}", bufs=4)
                nc.vector.tensor_tensor(out=lr[:, 0:gn, :],
                                        in0=ev2[:, 0:gn, :],
                                        in1=ev[:, 0:gn, :], op=ALU.max)
                exfb = sb.tile([P, G_GATHER, 2], bf16, tag=f"exfb{h}",
                               bufs=4)
                nc.scalar.activation(exfb[:, 0:gn, :], lr[:, 0:gn, :],
                                     AF.Exp)
                # batched S0/S1 for the whole group: one DVE op per head
                span = gat_S(h, g0, span=gn)     # [P, gn*128]
                s3 = span.rearrange("p (g c) -> p g c", g=gn)
                S0g = sb.tile([P, G_GATHER, P], bf16, tag=f"S0g{h}",
                              bufs=4)
                nc.vector.tensor_tensor(
                    out=S0g[:, 0:gn, :], in0=s3,
                    in1=exfb[:, 0:gn, 0:1].to_broadcast([P, gn, P]),
                    op=ALU.mult)
                S1g = sb.tile([P, G_GATHER, P], bf16, tag=f"S1g{h}",
                              bufs=4)
                nc.vector.tensor_tensor(
                    out=S1g[:, 0:gn, :], in0=s3,
                    in1=exfb[:, 0:gn, 1:2].to_broadcast([P, gn, P]),
                    op=ALU.mult)
                gstate[key] = (xg, S0g, S1g)

            def gat_tile(h, t):
                g = t // G_GATHER
                for pf in range(4):
                    gat_dma(h, g + pf)
                gat_chain(h, g)
                gat_chain(h, g + 1)
                xg, S0g, S1g = gstate[(h, g)]
                j = t - g * G_GATHER
                return xg[:, j, :], S0g[:, j, :], S1g[:, j, :]

            # warm chunk-A gathers into the Pool queue BEFORE the chunk-B
            # collective (the in-order Pool engine would otherwise hold
            # every GAT gather hostage behind ag1b's data dependency)
            for g_ in range(4):
                gat_dma(0, g_)
            sc2 = nc.enter_named_scope("ag1", False)
            nc.gpsimd.collective_compute(
                "AllGather", ALU.bypass, replica_groups=rg,
                ins=[cc1_in[1][:]], outs=[cc1_out[1][:]])
            nc.leave_named_scope("ag1", sc2[0], False)

            for b in range(B):
                vld = min(P, SH - b * P)
                r0 = b * P
                tiles = [(h, t) for h in range(2)
                         for t in range(int(gat["tile_off"][h, b]),
                                        int(gat["tile_off"][h, b + 1]))]
                p0 = ps.tile([P, 129], f32, tag="accA", bufs=2)
                p1 = ps.tile([P, 129], f32, tag="accB", bufs=1)
                for j, (h, t) in enumerate(tiles):
                    xg, S0, S1 = gat_tile(h, t)
                    nc.tensor.matmul(p0[:], lhsT=S0, rhs=xg[:, 0:129],
                                     start=(j == 0),
                                     stop=(j == len(tiles) - 1))
                    nc.tensor.matmul(p1[:], lhsT=S1, rhs=xg[:, 0:129],
                                     start=(j == 0),
                                     stop=(j == len(tiles) - 1))
                # ---- block flush: normalize, apply W_gat ----
                rec = sb.tile([P, 2], f32, tag="rec")
                nc.vector.reciprocal(rec[:, 0:1], p0[:, 128:129])
                nc.vector.reciprocal(rec[:, 1:2], p1[:, 128:129])
                nm = []
                for hh, pp_ in ((0, p0), (1, p1)):
                    nmh = sb.tile([P, P], bf16, tag=f"nm{hh}")
                    nc.vector.tensor_tensor(
                        out=nmh[:], in0=pp_[:, 0:128],
                        in1=rec[:, hh:hh + 1].to_broadcast([P, P]),
                        op=ALU.mult)
                    ptn = ps.tile([P, P], bf16, tag="mm", bufs=2)
                    nc.tensor.transpose(ptn[:], nmh[:], identb[:])
                    tb = sb.tile([P, P], bf16, tag=f"tb{hh}")
                    nc.scalar.copy(tb[:], ptn[:])
                    nm.append(tb)
                px2 = ps.tile([P, P], f32, tag="mm", bufs=2)
                nc.tensor.matmul(px2[:], lhsT=wg0_sb[:], rhs=nm[0][:],
                                 start=True, stop=False)
                nc.tensor.matmul(px2[:], lhsT=wg1_sb[:], rhs=nm[1][:],
                                 start=False, stop=True)
                x2T = sb.tile([P, P], f32, tag="x2T")
                nc.scalar.activation(x2T[:], px2[:], AF.Relu,
                                     bias=bg_sb[:, 0:1], scale=0.5)
                nc.scalar.copy(x2Tb_sb[:, r0:r0 + P], x2T[:])
                ptx = ps.tile([P, P], bf16, tag="mm", bufs=2)
                nc.tensor.transpose(ptx[:], x2Tb_sb[:, r0:r0 + P],
                                    identb[:])
                x2n = sb.tile([P, P], f32, tag="x2n")
                nc.scalar.copy(x2n[:], ptx[:])
                x2nb = sb.tile([P, P], bf16, tag="x2nb")
                nc.vector.tensor_copy(x2nb[:], ptx[:])
                hc, rc = (0, r0) if b < BCUT else (1, r0 - CH)
                nc.sync.dma_start(cc2_in[hc][rc:rc + vld, :], x2nb[:vld, :])
                nc.sync.dma_start(out_dram[r0:r0 + vld, D:2 * D],
                                  x2n[:vld, :])
                if b == BCUT + 2:
                    nc.gpsimd.collective_compute(
                        "AllGather", ALU.bypass, replica_groups=rg,
                        ins=[cc2_in[0][:]], outs=[cc2_out[0][:]])
            nc.leave_named_scope("gat", sc[0], False)

            # =================== Stage 3: RGCN ===================
            sc = nc.enter_named_scope("rgcn", False)
            rgcn_xg = make_gather(
                "r", rgcn, (cc2_out[0][:], cc2_out[1][:]), D)
            rgcn_xg(0, 0)     # warm chunk-A gathers ahead of ag2b
            sc2 = nc.enter_named_scope("ag2", False)
            nc.gpsimd.collective_compute(
                "AllGather", ALU.bypass, replica_groups=rg,
                ins=[cc2_in[1][:]], outs=[cc2_out[1][:]])
            nc.leave_named_scope("ag2", sc2[0], False)
            RTAGS = {0: ("accA", 2), 1: ("accA", 2), 2: ("accB", 1),
                     3: ("accC", 2), 4: ("accC", 2)}
            for b in range(B):
                vld = min(P, SH - b * P)
                r0 = b * P
                # runs of this block in execution order
                bruns = []
                for h in range(2):
                    t0 = int(rgcn["tile_off"][h, b])
                    for tl in range(int(rgcn["sec_tiles"][h, b])):
                        t = t0 + tl
                        for r in range(R):
                            if (h, t, r) in rgcn["run_idx"]:
                                bruns.append((h, t, r,
                                              rgcn["run_idx"][(h, t, r)]))
                present = sorted({r for (_, _, r, _) in bruns})
                pr = {}
                first = {r: True for r in present}
                lastrun = {}
                for i, (h, t, r, ri) in enumerate(bruns):
                    lastrun[r] = i
                swcache = {}
                for i, (h, t, r, ri) in enumerate(bruns):
                    if r not in pr:
                        tg, nb = RTAGS[present.index(r)]
                        pr[r] = ps.tile([P, P], f32, name=f"pr{r}",
                                        tag=tg, bufs=nb)
                    xg = rgcn_xg(h, t)
                    if (h, t) not in swcache:
                        S01 = sb.tile([P, P], bf16, tag="S01", bufs=3)
                        nc.vector.tensor_tensor(
                            out=S01[:], in0=iota_f[:],
                            in1=idx_sb[f"rd{h}"][:, t:t + 1].to_broadcast(
                                [P, P]),
                            op=ALU.is_equal)
                        swcache[(h, t)] = S01
                    Swr = sb.tile([P, P], bf16, tag="Swr", bufs=4)
                    if i % 2 == 0:
                        nc.scalar.activation(
                            Swr[:], swcache[(h, t)][:], AF.Copy,
                            scale=idx_sb[f"rw{h}"][:, ri:ri + 1])
                    else:
                        nc.vector.tensor_tensor(
                            out=Swr[:], in0=swcache[(h, t)][:],
                            in1=idx_sb[f"rb{h}"][:, ri:ri + 1].to_broadcast(
                                [P, P]),
                            op=ALU.mult)
                    nc.tensor.matmul(pr[r][:], lhsT=xg, rhs=Swr[:],
                                     start=first[r], stop=(lastrun[r] == i))
                    first[r] = False
                px3 = ps.tile([P, P], f32, tag="px", bufs=1)
                nc.tensor.matmul(px3[:], lhsT=wroot_sb[:],
                                 rhs=x2Tb_sb[:, r0:r0 + P],
                                 start=True, stop=(not present))
                for i, r in enumerate(present):
                    aggTb = sb.tile([P, P], bf16, tag="raggTb")
                    nc.scalar.copy(aggTb[:], pr[r][:])
                    nc.tensor.matmul(px3[:], lhsT=wr_sb[:, r * D:(r + 1) * D],
                                     rhs=aggTb[:], start=False,
                                     stop=(i == len(present) - 1))
                x3T = sb.tile([P, P], f32, tag="x3T")
                nc.scalar.activation(x3T[:], px3[:], AF.Identity,
                                     bias=br_sb[:, 0:1])
                ptr3 = ps.tile([P, P], f32, tag="mm", bufs=2)
                nc.tensor.transpose(ptr3[:], x3T[:], ident[:])
                x3n = sb.tile([P, P], f32, tag="x3n")
                nc.scalar.copy(x3n[:], ptr3[:])
                nc.sync.dma_start(out_dram[r0:r0 + vld, 2 * D:3 * D],
                                  x3n[:vld, :])
            nc.leave_named_scope("rgcn", sc[0], False)

    nc.compile()
    return nc


def kernel(x, edge_index, edge_type, W_sage_l, b_sage, W_sage_r,
           W_gat, att_src, att_dst, b_gat, W_rgcn, W_root, b_rgcn,
           _trace=False, _tmpdir=None):
    x = np.asarray(x, np.float32)
    edge_index = np.asarray(edge_index)
    edge_type = np.asarray(edge_type)

    pp = _preprocess(x, edge_index, edge_type)
    nc = _build_program(pp)

    W_gat = np.asarray(W_gat, np.float32)
    v = np.empty((D, 4), np.float32)
    for h in range(H):
        v[:, h] = W_gat[:, h, :] @ np.asarray(att_src, np.float32)[h]
        v[:, 2 + h] = W_gat[:, h, :] @ np.asarray(att_dst, np.float32)[h]

    common = {
        "wsl": np.asarray(W_sage_l, np.float32).astype(BF),
        "wsr": np.asarray(W_sage_r, np.float32).astype(BF),
        "bs": np.asarray(b_sage, np.float32).reshape(P, 1),
        "vsd": v,
        "wg0": W_gat[:, 0, :].astype(BF),
        "wg1": W_gat[:, 1, :].astype(BF),
        "bg": np.asarray(b_gat, np.float32).reshape(P, 1),
        "wroot": np.asarray(W_root, np.float32).astype(BF),
        "wr": np.asarray(W_rgcn, np.float32).astype(BF),
        "br": np.asarray(b_rgcn, np.float32).reshape(P, 1),
    }

    in_maps = []
    for k in range(NCORES):
        xs = np.zeros((B * P, D), np.float32)
        xs[:SH] = x[k * SH:(k + 1) * SH]
        m = dict(common)
        m["xt"] = np.ascontiguousarray(
            xs.reshape(B, P, D).transpose(0, 2, 1)).astype(BF)
        for st, d_ in (("e", pp["sage"]), ("g", pp["gat"]),
                       ("r", pp["rgcn"])):
            for h in range(2):
                if d_["T"][h] == 0:
                    continue
                ch = d_["cores"][k][h]
                if st == "e":
                    # host pre-gather with folded mean weight
                    rows = (x[ch["srcs"]] *
                            ch["ws"][:, None]).astype(BF)
                    m[f"ex{h}"] = _to_pm_g(rows, d_["T"][h], 16)
                    m[f"ed{h}"] = ch["dcol"]
                elif st == "g":
                    m[f"gi{h}"] = ch["idx16"]
                    m[f"gs{h}"] = _to_pm_g(ch["S_flat"], d_["T"][h], G_S)
                    m[f"gq{h}"] = _to_pm_g(ch["ST_flat"], d_["T"][h], G_S)
                else:
                    m[f"ri{h}"] = ch["idx16"]
                    m[f"rd{h}"] = ch["dcol"]
                    m[f"rw{h}"] = ch["rwcol"]
                    m[f"rb{h}"] = ch["rwcol"].astype(BF)
        in_maps.append(m)

    res = run_bass_kernel_spmd(nc, in_maps, core_ids=list(range(NCORES)),
                               trace=_trace, tmpdir=_tmpdir)
    out = np.concatenate([res.results[k]["out"] for k in range(NCORES)], 0)
    if _trace:
        return out, res
    return out


# revision 92
# speedup vs baseline: 1.0965x; 1.0965x over previous
"""Trainium2 Bass kernel for ClassForgeEnsembleGNN (SAGE -> GAT -> RGCN ensemble).

Strategy (8 NeuronCores, SPMD):
  - Nodes partitioned into 8 contiguous shards (6250 each); each core owns the
    edges whose target is in its shard.  Weights replicated; x1/x2 node
    features all-gathered between stages (device collectives).
  - Per-edge source rows fetched with batched GPSIMD dma_gather (int16
    indices; tables addressed through two views split at 32768).
  - Scatter-aggregation via selection-matrix matmuls where the selection
    matrices S[e, n] (0/1, with 1/deg mean weights folded in bf16) are
    PRECOMPUTED ON HOST and streamed from HBM in 512KB group DMAs -- no
    on-device S construction.
  - SAGE/RGCN aggregate feature-major (lhsT=gathered rows, rhs=S) so the
    result lands [d, n] ready for the weight matmul without a transpose.
  - GAT: self-loops appended as real edges; per-edge a_d via a transposed
    selection matmul (S_T also precomputed); exp(leaky(logit)) folded into
    S at runtime by one DVE scale per head; denominator via a ones column.
  - RGCN: relation runs inside tiles use per-run masked S; all 5 relation
    terms + root accumulate into one PSUM tile per block.
"""

import sys
import os

for _p in ("/opt/trn_rl_repo", "/root/.axon_site/_ro/trn_rl_repo"):
    if os.path.isdir(_p) and _p not in sys.path:
        sys.path.append(_p)

import numpy as np
import ml_dtypes

import concourse.bacc as bacc
import concourse.bass as bass
import concourse.mybir as mybir
import concourse.tile as tile
from concourse.bass_utils import run_bass_kernel_spmd
from concourse.masks import make_identity

P = 128
NCORES = 8
N = 50000
E = 400000
D = 128
H = 2
R = 5
NEG = 0.2
SH = N // NCORES            # 6250 nodes per shard
B = (SH + P - 1) // P       # 49 blocks (last has 106 valid nodes)
HSPLIT = 32768              # int16-safe table split
BCUT = 25                   # blocks in collective chunk A
CH = BCUT * P               # 3200 rows per shard in chunk A
CB = SH - CH                # 3050 rows in chunk B
G_GATHER = 8                # tiles per dma_gather group
G_S = 16                    # tiles per S-matrix group DMA

f32 = mybir.dt.float32
bf16 = mybir.dt.bfloat16
i32 = mybir.dt.int32
i16 = mybir.dt.int16
AF = mybir.ActivationFunctionType
ALU = mybir.AluOpType
BF = ml_dtypes.bfloat16


def _pack_idx(idx, T):
    """flat slot->src indices [T*128] -> dma_gather int16 layout [128, T*8]."""
    m = idx.reshape(-1, 16).T.astype(np.int16)
    return np.ascontiguousarray(np.tile(m, (8, 1)))


def _to_pm(S_flat, T):
    """[T*128, 128] -> partition-major [128, T*128] (tile t at cols t*128:)."""
    return np.ascontiguousarray(
        S_flat.reshape(T, P, P).transpose(1, 0, 2).reshape(P, T * P))


def _to_pm_g(S_flat, T, G):
    """[T*128, 128] -> group-contiguous [nG, 128, G*128]: each group's
    partition-major block is one linear DRAM region (max DMA bandwidth)."""
    nG = (T + G - 1) // G
    S = np.zeros((nG * G * P, P), S_flat.dtype)
    S[:T * P] = S_flat
    return np.ascontiguousarray(
        S.reshape(nG, G, P, P).transpose(0, 2, 1, 3).reshape(nG, P, G * P))


def _pack_stage(half_pc, tidx_pc, src_pc, dst_pc, w_pc, rel_pc, nrel):
    """Group edges by (half, dst_block [, rel]); sections (h, b) tile-aligned,
    rel runs slot-aligned; uniform tile/slot caps across cores.

    half_pc: per-edge stream id (0/1); tidx_pc: per-edge gather-table index
    within its stream.
    """
    counts = np.zeros((NCORES, 2, B, nrel), np.int64)
    for k in range(NCORES):
        h = half_pc[k].astype(np.int64)
        blk = dst_pc[k] // P
        r = rel_pc[k] if nrel > 1 else np.zeros(len(src_pc[k]), np.int64)
        np.add.at(counts[k], (h, blk, r), 1)
    caps = counts.max(0)                       # [2, B, nrel]
    run_off = np.zeros((2, B, nrel + 1), np.int64)
    np.cumsum(caps, axis=2, out=run_off[:, :, 1:])
    sec_slots = run_off[:, :, nrel]
    sec_tiles = (sec_slots + P - 1) // P
    tile_off = np.zeros((2, B + 1), np.int64)
    np.cumsum(sec_tiles, axis=1, out=tile_off[:, 1:])
    T = (int(tile_off[0, B]), int(tile_off[1, B]))

    # runs: list per h of (b, tile_in_stream, r, slot_lo, slot_hi) in
    # execution order (b asc, tile asc, r asc); only for nrel > 1
    runs = ([], [])
    run_idx = {}
    if nrel > 1:
        for b in range(B):
            for h in range(2):
                t0 = int(tile_off[h, b])
                for tl in range(int(sec_tiles[h, b])):
                    s0, s1 = tl * P, (tl + 1) * P
                    for r in range(nrel):
                        lo = int(run_off[h, b, r])
                        hi = int(run_off[h, b, r + 1])
                        if lo < s1 and hi > s0:
                            run_idx[(h, t0 + tl, r)] = len(runs[h])
                            runs[h].append((b, t0 + tl, r,
                                            max(lo, s0) - s0,
                                            min(hi, s1) - s0))

    cores = []
    for k in range(NCORES):
        src = src_pc[k]
        tidx = tidx_pc[k]
        dst = dst_pc[k]
        w = w_pc[k]
        h = half_pc[k].astype(np.int64)
        blk = dst // P
        r = rel_pc[k] if nrel > 1 else np.zeros(len(src), np.int64)
        gid = (h * B + blk) * nrel + r
        order = np.argsort(gid * np.int64(N) + src, kind="stable")
        gs = gid[order]
        cnt_flat = counts[k].reshape(-1)
        starts = np.concatenate([[0], np.cumsum(cnt_flat)])[:-1]
        rank = np.arange(len(gs)) - starts[gs]
        hh, rest = gs // (B * nrel), gs % (B * nrel)
        bb, rr = rest // nrel, rest % nrel
        slot = tile_off[hh, bb] * P + run_off[hh, bb, rr] + rank
        per_h = []
        for hv in range(2):
            Th = T[hv]
            n_slots = Th * P
            sel = hh == hv
            s = slot[sel]
            idx = np.zeros(n_slots, np.int64)
            idx[s] = tidx[order][sel]
            srcs = np.zeros(n_slots, np.int64)
            srcs[s] = src[order][sel]
            ws = np.zeros(n_slots, np.float32)
            ws[s] = w[order][sel]
            dcol = np.full(n_slots, -1.0, np.float32)
            dcol[s] = (dst[order][sel] % P).astype(np.float32)
            if nrel > 1:
                # per-run weight column: w inside the run's slot range,
                # 0 elsewhere (masks the tile-shared 0/1 S to this run)
                NR = len(runs[hv])
                rwc = np.zeros((NR, P), np.float32)
                for j, (b_, t_, r_, lo_, hi_) in enumerate(runs[hv]):
                    rwc[j, lo_:hi_] = ws[t_ * P + lo_:t_ * P + hi_]
                rwcol = np.ascontiguousarray(rwc.T)
            else:
                rwcol = None
            S_flat = np.zeros((n_slots, P), np.float32)
            S_flat[s, (dst[order][sel] % P)] = w[order][sel]
            per_h.append(dict(idx16=_pack_idx(idx, Th), srcs=srcs,
                              ws=ws,
                              dcol=np.ascontiguousarray(
                                  dcol.reshape(Th, P).T) if Th else None,
                              rwcol=rwcol,
                              S_flat=S_flat.astype(BF) if nrel == 1
                              else None))
        cores.append(per_h)

    return dict(T=T, tile_off=tile_off, sec_tiles=sec_tiles, runs=runs,
                run_idx=run_idx, cores=cores)


def _preprocess(x, edge_index, edge_type):
    src = edge_index[0].astype(np.int64)
    dst = edge_index[1].astype(np.int64)
    et = edge_type.astype(np.int64)

    cnt = np.bincount(dst, minlength=N).astype(np.float32)
    wrec = (1.0 / np.maximum(cnt, 1.0)).astype(np.float32)
    cnt_r = np.zeros((R, N), np.float32)
    for r in range(R):
        cnt_r[r] = np.bincount(dst[et == r], minlength=N)
    wrrec = (1.0 / np.maximum(cnt_r, 1.0)).astype(np.float32)

    def chunk_of(s):
        """Stream (0=A,1=B) and table index for the chunked AllGather
        layout: chunk A = first CH rows of each shard, B = the rest."""
        sh, loc = s // SH, s % SH
        half = (loc >= CH).astype(np.int64)
        tidx = np.where(half == 0, sh * CH + loc, sh * CB + (loc - CH))
        return half, tidx

    shard_of = dst // SH
    s_src, s_dst, s_rel, s_w, s_wr, s_h, s_t = [], [], [], [], [], [], []
    g_src, g_dst, g_h, g_t = [], [], [], []
    for k in range(NCORES):
        sel = shard_of == k
        s_src.append(src[sel])
        s_dst.append(dst[sel] - k * SH)
        s_rel.append(et[sel])
        s_w.append(wrec[dst[sel]])
        s_wr.append(wrrec[et[sel], dst[sel]])
        s_h.append(np.zeros(sel.sum(), np.int64))
        s_t.append(src[sel])
        hh, tt = chunk_of(src[sel])
        # GAT: append self loops (weight 1.0 folded at runtime with exp)
        loc = np.arange(SH, dtype=np.int64)
        gsrc = np.concatenate([src[sel], loc + k * SH])
        g_src.append(gsrc)
        g_dst.append(np.concatenate([dst[sel] - k * SH, loc]))
        ghh, gtt = chunk_of(gsrc)
        g_h.append(ghh)
        g_t.append(gtt)

    sage = _pack_stage(s_h, s_t, s_src, s_dst, s_w,
                       [None] * NCORES, 1)
    gat = _pack_stage(g_h, g_t, g_src, g_dst,
                      [np.ones(len(g_src[k]), np.float32)
                       for k in range(NCORES)],
                      [None] * NCORES, 1)
    rgcn = _pack_stage([chunk_of(s)[0] for s in s_src],
                       [chunk_of(s)[1] for s in s_src],
                       s_src, s_dst, s_wr, s_rel, R)

    # GAT transposed selection (0/1): ST[t][n, e] = S[t][e, n]
    for k in range(NCORES):
        for hv in range(2):
            Th = gat["T"][hv]
            Sf = gat["cores"][k][hv]["S_flat"]       # [T*128, 128]
            STf = Sf.reshape(Th, P, P).transpose(0, 2, 1).reshape(Th * P, P)
            gat["cores"][k][hv]["ST_flat"] = STf

    return dict(sage=sage, gat=gat, rgcn=rgcn)


def _build_program(pp):
    sage, gat, rgcn = pp["sage"], pp["gat"], pp["rgcn"]

    nc = bacc.Bacc("TRN2", target_bir_lowering=False, debug=False,
                   num_devices=NCORES, num_swdge_queues=4)

    xt_dram = nc.dram_tensor("xt", [B, P, P], bf16, kind="ExternalInput")
    meta = {}
    for st, d_ in (("e", sage), ("g", gat), ("r", rgcn)):
        for h in range(2):
            T = d_["T"][h]
            if T == 0:
                continue
            if st == "e":
                # SAGE source rows pre-gathered on host (x is an input),
                # with the 1/deg mean weight folded in; S built on device
                # from the dst column.
                meta[f"ex{h}"] = nc.dram_tensor(
                    f"ex{h}", [(T + 31) // 32, P, 32 * P], bf16,
                    kind="ExternalInput")
                meta[f"ed{h}"] = nc.dram_tensor(
                    f"ed{h}", [P, T], f32, kind="ExternalInput")
            elif st == "g":
                meta[f"gi{h}"] = nc.dram_tensor(
                    f"gi{h}", [P, T * 8], i16, kind="ExternalInput")
                meta[f"gs{h}"] = nc.dram_tensor(
                    f"gs{h}", [(T + G_S - 1) // G_S, P, G_S * P], bf16,
                    kind="ExternalInput")
                meta[f"gq{h}"] = nc.dram_tensor(
                    f"gq{h}", [(T + G_S - 1) // G_S, P, G_S * P], bf16,
                    kind="ExternalInput")
            else:
                meta[f"ri{h}"] = nc.dram_tensor(
                    f"ri{h}", [P, T * 8], i16, kind="ExternalInput")
                meta[f"rd{h}"] = nc.dram_tensor(
                    f"rd{h}", [P, T], f32, kind="ExternalInput")
                meta[f"rw{h}"] = nc.dram_tensor(
                    f"rw{h}", [P, len(d_["runs"][h])], f32,
                    kind="ExternalInput")
                meta[f"rb{h}"] = nc.dram_tensor(
                    f"rb{h}", [P, len(d_["runs"][h])], bf16,
                    kind="ExternalInput")
    wsl = nc.dram_tensor("wsl", [D, D], bf16, kind="ExternalInput")
    wsr = nc.dram_tensor("wsr", [D, D], bf16, kind="ExternalInput")
    bs = nc.dram_tensor("bs", [P, 1], f32, kind="ExternalInput")
    vsd = nc.dram_tensor("vsd", [D, 4], f32, kind="ExternalInput")
    wg0 = nc.dram_tensor("wg0", [D, D], bf16, kind="ExternalInput")
    wg1 = nc.dram_tensor("wg1", [D, D], bf16, kind="ExternalInput")
    bg = nc.dram_tensor("bg", [P, 1], f32, kind="ExternalInput")
    wroot = nc.dram_tensor("wroot", [D, D], bf16, kind="ExternalInput")
    wr_d = nc.dram_tensor("wr", [R, D, D], bf16, kind="ExternalInput")
    br = nc.dram_tensor("br", [P, 1], f32, kind="ExternalInput")
    out_dram = nc.dram_tensor("out", [SH, 3 * D], f32, kind="ExternalOutput")

    rg = [list(range(NCORES))]
    qrr = [0]

    def next_q():
        q = qrr[0]
        qrr[0] = (q + 1) % 4
        return q

    with tile.TileContext(nc) as tc:
        with (
            tc.tile_pool(name="const", bufs=1) as cb,
            tc.tile_pool(name="sbuf", bufs=2) as sb,
            tc.tile_pool(name="psum", bufs=1, space="PSUM") as ps,
            tc.tile_pool(name="dram", bufs=1, space="DRAM") as dr,
        ):
            ident = cb.tile([P, P], f32)
            make_identity(nc, ident[:])
            identb = cb.tile([P, P], bf16)
            nc.vector.tensor_copy(identb[:], ident[:])
            iota_i = cb.tile([P, P], i32)
            nc.gpsimd.iota(iota_i[:], pattern=[[1, P]], base=0,
                           channel_multiplier=0)
            iota_f = cb.tile([P, P], f32)
            nc.vector.tensor_copy(iota_f[:], iota_i[:])

            def load_const(name, dram, dtype):
                t = cb.tile(list(dram.shape), dtype, name=name)
                nc.sync.dma_start(t[:], dram[:])
                return t

            wsl_sb = load_const("wsl_sb", wsl, bf16)
            wsr_sb = load_const("wsr_sb", wsr, bf16)
            bs_sb = load_const("bs_sb", bs, f32)
            vsd_sb = load_const("vsd_sb", vsd, f32)
            wg0_sb = load_const("wg0_sb", wg0, bf16)
            wg1_sb = load_const("wg1_sb", wg1, bf16)
            bg_sb = load_const("bg_sb", bg, f32)
            wroot_sb = load_const("wroot_sb", wroot, bf16)
            br_sb = load_const("br_sb", br, f32)
            wr_sb = cb.tile([P, R * D], bf16)
            for r in range(R):
                nc.sync.dma_start(wr_sb[:, r * D:(r + 1) * D], wr_d[r, :, :])
            idx_sb = {}
            for name, dram in meta.items():
                if name[1] not in "idwb" or name == "xb":
                    continue
                t = cb.tile(list(dram.shape), dram.dtype, name=f"{name}_sb")
                nc.sync.dma_start(t[:], dram[:])
                idx_sb[name] = t

            # persistent per-shard state
            adb_sb = cb.tile([P, 2 * B], bf16)   # a_d bf16 per block
            x2Tb_sb = cb.tile([P, B * P], bf16)  # x2 feature-major bf16

            cc1_in = [dr.tile([CH, 256], bf16, name="cc1a_in"),
                      dr.tile([CB, 256], bf16, name="cc1b_in")]
            cc1_out = [dr.tile([NCORES * CH, 256], bf16,
                               addr_space="Shared", name="cc1a_out"),
                       dr.tile([NCORES * CB, 256], bf16,
                               addr_space="Shared", name="cc1b_out")]
            cc2_in = [dr.tile([CH, D], bf16, name="cc2a_in"),
                      dr.tile([CB, D], bf16, name="cc2b_in")]
            cc2_out = [dr.tile([NCORES * CH, D], bf16,
                               addr_space="Shared", name="cc2a_out"),
                       dr.tile([NCORES * CB, D], bf16,
                               addr_space="Shared", name="cc2b_out")]

            one3 = cb.tile([P, 1], bf16)
            nc.vector.memset(one3[:], 1.0)
            negc = cb.tile([P, 1], f32)
            nc.vector.memset(negc[:], NEG)

            # ---------- streamed gather groups (prefetch 1 ahead) ----------
            def make_gather(st, d_, table_views, width):
                state = {}

                def load(h, g):
                    key = (h, g)
                    if key in state:
                        return
                    T = d_["T"][h]
                    g0 = g * G_GATHER
                    if g0 >= T:
                        return
                    gn = min(G_GATHER, T - g0)
                    xg = sb.tile([P, G_GATHER, width], bf16,
                                 tag=f"{st}xg{h}", bufs=5)
                    ni = gn * P
                    nc.gpsimd.dma_gather(
                        xg[:, 0:gn, :], table_views[h],
                        idx_sb[f"{st}i{h}"][:, g0 * 8:(g0 + gn) * 8],
                        ni, ni, width, queue_num=next_q())
                    state[key] = xg

                def get_tile(h, t):
                    g = t // G_GATHER
                    for pf in range(4):
                        load(h, g + pf)
                    return state[(h, g)][:, t - g * G_GATHER, :]

                return get_tile

            # ---------- streamed S-matrix groups ----------
            def make_sstream(name, nmax_by_h, gs=G_S, bufs=2, pf=1):
                state = {}

                def load(h, g):
                    key = (h, g)
                    if key in state:
                        return
                    nmax = nmax_by_h[h]
                    g0 = g * gs
                    if g0 >= nmax:
                        return
                    gn = min(gs, nmax - g0)
                    t_ = sb.tile([P, gs * P], bf16, tag=f"{name}{h}",
                                 bufs=bufs)
                    nc.scalar.dma_start(
                        t_[:, 0:gn * P],
                        meta[f"{name}{h}"][g, :, 0:gn * P])
                    state[key] = t_

                def get(h, j, span=1):
                    g = j // gs
                    for q in range(pf + 1):
                        load(h, g + q)
                    j0 = j - g * gs
                    return state[(h, g)][:, j0 * P:(j0 + span) * P]

                return get

            # =================== Stage 1: SAGE ===================
            sc = nc.enter_named_scope("sage", False)
            sage_xg = make_sstream("ex", sage["T"], gs=32)
            for b in range(B):
                vld = min(P, SH - b * P)
                r0 = b * P
                tiles = [(0, t) for t in range(int(sage["tile_off"][0, b]),
                                               int(sage["tile_off"][0, b + 1]))]
                pa = ps.tile([P, P], f32, tag="accA", bufs=2)
                if tiles:
                    for j, (h, t) in enumerate(tiles):
                        xg = sage_xg(h, t)
                        Sw = sb.tile([P, P], bf16, tag="Ssage", bufs=4)
                        nc.vector.tensor_tensor(
                            out=Sw[:], in0=iota_f[:],
                            in1=idx_sb["ed0"][:, t:t + 1].to_broadcast(
                                [P, P]),
                            op=ALU.is_equal)
                        nc.tensor.matmul(pa[:], lhsT=xg, rhs=Sw[:],
                                         start=(j == 0),
                                         stop=(j == len(tiles) - 1))
                else:
                    nc.vector.memset(pa[:], 0.0)
                aggTb = sb.tile([P, P], bf16, tag="aggTb")
                nc.scalar.copy(aggTb[:], pa[:])
                xT = sb.tile([P, P], bf16, tag="xT")
                nc.scalar.dma_start(xT[:], xt_dram[b, :, :])
                pm = ps.tile([P, P], f32, tag="mm", bufs=2)
                nc.tensor.matmul(pm[:], lhsT=wsl_sb[:], rhs=aggTb[:],
                                 start=True, stop=False)
                nc.tensor.matmul(pm[:], lhsT=wsr_sb[:], rhs=xT[:],
                                 start=False, stop=True)
                x1T = sb.tile([P, P], f32, tag="x1T")
                nc.scalar.activation(x1T[:], pm[:], AF.Relu,
                                     bias=bs_sb[:, 0:1], scale=1.0)
                pasd = ps.tile([P, 4], f32, tag="px", bufs=1)
                nc.tensor.matmul(pasd[:], lhsT=x1T[:], rhs=vsd_sb[:],
                                 start=True, stop=True)
                nc.scalar.copy(adb_sb[:, 2 * b:2 * b + 2], pasd[:, 2:4])
                ase = sb.tile([P, 3], bf16, tag="ase")
                nc.vector.tensor_copy(ase[:, 0:1], one3[:])
                nc.vector.tensor_copy(ase[:, 1:3], pasd[:, 0:2])
                ptr = ps.tile([P, P], f32, tag="mm", bufs=2)
                nc.tensor.transpose(ptr[:], x1T[:], ident[:])
                x1n = sb.tile([P, P], f32, tag="x1n")
                nc.scalar.copy(x1n[:], ptr[:])
                x1nb = sb.tile([P, P], bf16, tag="x1nb")
                nc.vector.tensor_copy(x1nb[:], ptr[:])
                hc, rc = (0, r0) if b < BCUT else (1, r0 - CH)
                nc.sync.dma_start(cc1_in[hc][rc:rc + vld, 0:D],
                                  x1nb[:vld, :])
                nc.sync.dma_start(cc1_in[hc][rc:rc + vld, D:D + 3],
                                  ase[:vld, :])
                nc.sync.dma_start(out_dram[r0:r0 + vld, 0:D], x1n[:vld, :])
                if b == BCUT - 1:
                    nc.gpsimd.collective_compute(
                        "AllGather", ALU.bypass, replica_groups=rg,
                        ins=[cc1_in[0][:]], outs=[cc1_out[0][:]])
            nc.leave_named_scope("sage", sc[0], False)

            # =================== Stage 2: GAT ===================
            sc = nc.enter_named_scope("gat", False)
            gat_S = make_sstream("gs", gat["T"])
            gat_ST = make_sstream("gq", gat["T"])

            # block of each gat tile (for adb lookup at group level)
            gat_blk = [np.zeros(max(gat["T"][h], 1), np.int64)
                       for h in range(2)]
            for h in range(2):
                for b in range(B):
                    gat_blk[h][int(gat["tile_off"][h, b]):
                               int(gat["tile_off"][h, b + 1])] = b

            # GAT gather groups: fetch rows + compute per-edge exp(leaky)
            # attention factors for the whole group in O(1) instructions.
            gdma = {}
            gstate = {}

            def gat_dma(h, g):
                key = (h, g)
                if key in gdma:
                    return
                T = gat["T"][h]
                g0 = g * G_GATHER
                if g0 >= T:
                    return
                gn = min(G_GATHER, T - g0)
                xg = sb.tile([P, G_GATHER, 256], bf16,
                             tag=f"gxg{h}", bufs=5)
                ni = gn * P
                nc.gpsimd.dma_gather(
                    xg[:, 0:gn, :], cc1_out[h][:],
                    idx_sb[f"gi{h}"][:, g0 * 8:(g0 + gn) * 8],
                    ni, ni, 256, queue_num=next_q())
                gdma[key] = (xg, gn)

            def gat_chain(h, g):
                key = (h, g)
                if key in gstate:
                    return
                T = gat["T"][h]
                g0 = g * G_GATHER
                if g0 >= T:
                    return
                gat_dma(h, g)
                xg, gn = gdma[key]
                padg = ps.tile([P, G_GATHER, 2], f32, tag="accC", bufs=2)
                for jg in range(gn):
                    bb = int(gat_blk[h][g0 + jg])
                    nc.tensor.matmul(padg[:, jg, :],
                                     lhsT=gat_ST(h, g0 + jg),
                                     rhs=adb_sb[:, 2 * bb:2 * bb + 2],
                                     start=True, stop=True)
                asg = sb.tile([P, G_GATHER, 2], f32, tag=f"asg{h}", bufs=4)
                nc.scalar.copy(asg[:, 0:gn, :], xg[:, 0:gn, 129:131])
                ev = sb.tile([P, G_GATHER, 2], f32, tag=f"ev{h}", bufs=4)
                nc.vector.tensor_add(ev[:, 0:gn, :], asg[:, 0:gn, :],
                                     padg[:, 0:gn, :])
                ev2 = sb.tile([P, G_GATHER, 2], f32, tag=f"ev2{h}", bufs=4)
                nc.vector.tensor_tensor(
                    out=ev2[:, 0:gn, :], in0=ev[:, 0:gn, :],
                    in1=negc[:, 0:1].to_broadcast([P, gn, 2]),
                    op=ALU.mult)
                lr = sb.tile([P, G_GATHER, 2], f32, tag=f"lr{h}", bufs=4)
                nc.vector.tensor_tensor(out=lr[:, 0:gn, :],
                                        in0=ev2[:, 0:gn, :],
                                        in1=ev[:, 0:gn, :], op=ALU.max)
                exfb = sb.tile([P, G_GATHER, 2], bf16, tag=f"exfb{h}",
                               bufs=4)
                nc.scalar.activation(exfb[:, 0:gn, :], lr[:, 0:gn, :],
                                     AF.Exp)
                # batched S0/S1 for the whole group: one DVE op per head
                span = gat_S(h, g0, span=gn)     # [P, gn*128]
                s3 = span.rearrange("p (g c) -> p g c", g=gn)
                S0g = sb.tile([P, G_GATHER, P], bf16, tag=f"S0g{h}",
                              bufs=3)
                nc.vector.tensor_tensor(
                    out=S0g[:, 0:gn, :], in0=s3,
                    in1=exfb[:, 0:gn, 0:1].to_broadcast([P, gn, P]),
                    op=ALU.mult)
                S1g = sb.tile([P, G_GATHER, P], bf16, tag=f"S1g{h}",
                              bufs=3)
                nc.vector.tensor_tensor(
                    out=S1g[:, 0:gn, :], in0=s3,
                    in1=exfb[:, 0:gn, 1:2].to_broadcast([P, gn, P]),
                    op=ALU.mult)
                gstate[key] = (xg, S0g, S1g)

            def gat_tile(h, t):
                g = t // G_GATHER
                for pf in range(4):
                    gat_dma(h, g + pf)
                gat_chain(h, g)
                gat_chain(h, g + 1)
                xg, S0g, S1g = gstate[(h, g)]
                j = t - g * G_GATHER
                return xg[:, j, :], S0g[:, j, :], S1g[:, j, :]

            # warm chunk-A gathers into the Pool queue BEFORE the chunk-B
            # collective (the in-order Pool engine would otherwise hold
            # every GAT gather hostage behind ag1b's data dependency)
            for g_ in range(4):
                gat_dma(0, g_)
            sc2 = nc.enter_named_scope("ag1", False)
            nc.gpsimd.collective_compute(
                "AllGather", ALU.bypass, replica_groups=rg,
                ins=[cc1_in[1][:]], outs=[cc1_out[1][:]])
            nc.leave_named_scope("ag1", sc2[0], False)

            for b in range(B):
                vld = min(P, SH - b * P)
                r0 = b * P
                tiles = [(h, t) for h in range(2)
                         for t in range(int(gat["tile_off"][h, b]),
                                        int(gat["tile_off"][h, b + 1]))]
                p0 = ps.tile([P, 129], f32, tag="accA", bufs=2)
                p1 = ps.tile([P, 129], f32, tag="accB", bufs=1)
                for j, (h, t) in enumerate(tiles):
                    xg, S0, S1 = gat_tile(h, t)
                    nc.tensor.matmul(p0[:], lhsT=S0, rhs=xg[:, 0:129],
                                     start=(j == 0),
                                     stop=(j == len(tiles) - 1))
                    nc.tensor.matmul(p1[:], lhsT=S1, rhs=xg[:, 0:129],
                                     start=(j == 0),
                                     stop=(j == len(tiles) - 1))
                # ---- block flush: normalize, apply W_gat ----
                rec = sb.tile([P, 2], f32, tag="rec")
                nc.vector.reciprocal(rec[:, 0:1], p0[:, 128:129])
                nc.vector.reciprocal(rec[:, 1:2], p1[:, 128:129])
                nm = []
                for hh, pp_ in ((0, p0), (1, p1)):
                    nmh = sb.tile([P, P], bf16, tag=f"nm{hh}")
                    nc.vector.tensor_tensor(
                        out=nmh[:], in0=pp_[:, 0:128],
                        in1=rec[:, hh:hh + 1].to_broadcast([P, P]),
                        op=ALU.mult)
                    ptn = ps.tile([P, P], bf16, tag="mm", bufs=2)
                    nc.tensor.transpose(ptn[:], nmh[:], identb[:])
                    tb = sb.tile([P, P], bf16, tag=f"tb{hh}")
                    nc.scalar.copy(tb[:], ptn[:])
                    nm.append(tb)
                px2 = ps.tile([P, P], f32, tag="mm", bufs=2)
                nc.tensor.matmul(px2[:], lhsT=wg0_sb[:], rhs=nm[0][:],
                                 start=True, stop=False)
                nc.tensor.matmul(px2[:], lhsT=wg1_sb[:], rhs=nm[1][:],
                                 start=False, stop=True)
                x2T = sb.tile([P, P], f32, tag="x2T")
                nc.scalar.activation(x2T[:], px2[:], AF.Relu,
                                     bias=bg_sb[:, 0:1], scale=0.5)
                nc.scalar.copy(x2Tb_sb[:, r0:r0 + P], x2T[:])
                ptx = ps.tile([P, P], bf16, tag="mm", bufs=2)
                nc.tensor.transpose(ptx[:], x2Tb_sb[:, r0:r0 + P],
                                    identb[:])
                x2n = sb.tile([P, P], f32, tag="x2n")
                nc.scalar.copy(x2n[:], ptx[:])
                x2nb = sb.tile([P, P], bf16, tag="x2nb")
                nc.vector.tensor_copy(x2nb[:], ptx[:])
                hc, rc = (0, r0) if b < BCUT else (1, r0 - CH)
                nc.sync.dma_start(cc2_in[hc][rc:rc + vld, :], x2nb[:vld, :])
                nc.sync.dma_start(out_dram[r0:r0 + vld, D:2 * D],
                                  x2n[:vld, :])
                if b == BCUT + 7:
                    nc.gpsimd.collective_compute(
                        "AllGather", ALU.bypass, replica_groups=rg,
                        ins=[cc2_in[0][:]], outs=[cc2_out[0][:]])
            nc.leave_named_scope("gat", sc[0], False)

            # =================== Stage 3: RGCN ===================
            sc = nc.enter_named_scope("rgcn", False)
            rgcn_xg = make_gather(
                "r", rgcn, (cc2_out[0][:], cc2_out[1][:]), D)
            rgcn_xg(0, 0)     # warm chunk-A gathers ahead of ag2b
            sc2 = nc.enter_named_scope("ag2", False)
            nc.gpsimd.collective_compute(
                "AllGather", ALU.bypass, replica_groups=rg,
                ins=[cc2_in[1][:]], outs=[cc2_out[1][:]])
            nc.leave_named_scope("ag2", sc2[0], False)
            RTAGS = {0: ("accA", 2), 1: ("accA", 2), 2: ("accB", 1),
                     3: ("accC", 2), 4: ("accC", 2)}
            for b in range(B):
                vld = min(P, SH - b * P)
                r0 = b * P
                # runs of this block in execution order
                bruns = []
                for h in range(2):
                    t0 = int(rgcn["tile_off"][h, b])
                    for tl in range(int(rgcn["sec_tiles"][h, b])):
                        t = t0 + tl
                        for r in range(R):
                            if (h, t, r) in rgcn["run_idx"]:
                                bruns.append((h, t, r,
                                              rgcn["run_idx"][(h, t, r)]))
                present = sorted({r for (_, _, r, _) in bruns})
                pr = {}
                first = {r: True for r in present}
                lastrun = {}
                for i, (h, t, r, ri) in enumerate(bruns):
                    lastrun[r] = i
                swcache = {}
                for i, (h, t, r, ri) in enumerate(bruns):
                    if r not in pr:
                        tg, nb = RTAGS[present.index(r)]
                        pr[r] = ps.tile([P, P], f32, name=f"pr{r}",
                                        tag=tg, bufs=nb)
                    xg = rgcn_xg(h, t)
                    if (h, t) not in swcache:
                        S01 = sb.tile([P, P], bf16, tag="S01", bufs=3)
                        nc.vector.tensor_tensor(
                            out=S01[:], in0=iota_f[:],
                            in1=idx_sb[f"rd{h}"][:, t:t + 1].to_broadcast(
                                [P, P]),
                            op=ALU.is_equal)
                        swcache[(h, t)] = S01
                    Swr = sb.tile([P, P], bf16, tag="Swr", bufs=4)
                    if i % 2 == 0:
                        nc.scalar.activation(
                            Swr[:], swcache[(h, t)][:], AF.Copy,
                            scale=idx_sb[f"rw{h}"][:, ri:ri + 1])
                    else:
                        nc.vector.tensor_tensor(
                            out=Swr[:], in0=swcache[(h, t)][:],
                            in1=idx_sb[f"rb{h}"][:, ri:ri + 1].to_broadcast(
                                [P, P]),
                            op=ALU.mult)
                    nc.tensor.matmul(pr[r][:], lhsT=xg, rhs=Swr[:],
                                     start=first[r], stop=(lastrun[r] == i))
                    first[r] = False
                px3 = ps.tile([P, P], f32, tag="px", bufs=1)
                nc.tensor.matmul(px3[:], lhsT=wroot_sb[:],
                                 rhs=x2Tb_sb[:, r0:r0 + P],
                                 start=True, stop=(not present))
                for i, r in enumerate(present):
                    aggTb = sb.tile([P, P], bf16, tag="raggTb")
                    nc.scalar.copy(aggTb[:], pr[r][:])
                    nc.tensor.matmul(px3[:], lhsT=wr_sb[:, r * D:(r + 1) * D],
                                     rhs=aggTb[:], start=False,
                                     stop=(i == len(present) - 1))
                x3T = sb.tile([P, P], f32, tag="x3T")
                nc.scalar.activation(x3T[:], px3[:], AF.Identity,
                                     bias=br_sb[:, 0:1])
                ptr3 = ps.tile([P, P], f32, tag="mm", bufs=2)
                nc.tensor.transpose(ptr3[:], x3T[:], ident[:])
                x3n = sb.tile([P, P], f32, tag="x3n")
                nc.scalar.copy(x3n[:], ptr3[:])
                nc.sync.dma_start(out_dram[r0:r0 + vld, 2 * D:3 * D],
                                  x3n[:vld, :])
            nc.leave_named_scope("rgcn", sc[0], False)

    nc.compile()
    return nc


def kernel(x, edge_index, edge_type, W_sage_l, b_sage, W_sage_r,
           W_gat, att_src, att_dst, b_gat, W_rgcn, W_root, b_rgcn,
           _trace=False, _tmpdir=None):
    x = np.asarray(x, np.float32)
    edge_index = np.asarray(edge_index)
    edge_type = np.asarray(edge_type)

    pp = _preprocess(x, edge_index, edge_type)
    nc = _build_program(pp)

    W_gat = np.asarray(W_gat, np.float32)
    v = np.empty((D, 4), np.float32)
    for h in range(H):
        v[:, h] = W_gat[:, h, :] @ np.asarray(att_src, np.float32)[h]
        v[:, 2 + h] = W_gat[:, h, :] @ np.asarray(att_dst, np.float32)[h]

    common = {
        "wsl": np.asarray(W_sage_l, np.float32).astype(BF),
        "wsr": np.asarray(W_sage_r, np.float32).astype(BF),
        "bs": np.asarray(b_sage, np.float32).reshape(P, 1),
        "vsd": v,
        "wg0": W_gat[:, 0, :].astype(BF),
        "wg1": W_gat[:, 1, :].astype(BF),
        "bg": np.asarray(b_gat, np.float32).reshape(P, 1),
        "wroot": np.asarray(W_root, np.float32).astype(BF),
        "wr": np.asarray(W_rgcn, np.float32).astype(BF),
        "br": np.asarray(b_rgcn, np.float32).reshape(P, 1),
    }

    in_maps = []
    for k in range(NCORES):
        xs = np.zeros((B * P, D), np.float32)
        xs[:SH] = x[k * SH:(k + 1) * SH]
        m = dict(common)
        m["xt"] = np.ascontiguousarray(
            xs.reshape(B, P, D).transpose(0, 2, 1)).astype(BF)
        for st, d_ in (("e", pp["sage"]), ("g", pp["gat"]),
                       ("r", pp["rgcn"])):
            for h in range(2):
                if d_["T"][h] == 0:
                    continue
                ch = d_["cores"][k][h]
                if st == "e":
                    # host pre-gather with folded mean weight
                    rows = (x[ch["srcs"]] *
                            ch["ws"][:, None]).astype(BF)
                    m[f"ex{h}"] = _to_pm_g(rows, d_["T"][h], 32)
                    m[f"ed{h}"] = ch["dcol"]
                elif st == "g":
                    m[f"gi{h}"] = ch["idx16"]
                    m[f"gs{h}"] = _to_pm_g(ch["S_flat"], d_["T"][h], G_S)
                    m[f"gq{h}"] = _to_pm_g(ch["ST_flat"], d_["T"][h], G_S)
                else:
                    m[f"ri{h}"] = ch["idx16"]
                    m[f"rd{h}"] = ch["dcol"]
                    m[f"rw{h}"] = ch["rwcol"]
                    m[f"rb{h}"] = ch["rwcol"].astype(BF)
        in_maps.append(m)

    res = run_bass_kernel_spmd(nc, in_maps, core_ids=list(range(NCORES)),
                               trace=_trace, tmpdir=_tmpdir)
    out = np.concatenate([res.results[k]["out"] for k in range(NCORES)], 0)
    if _trace:
        return out, res
    return out
